# revision 1
# baseline (speedup 1.0000x reference)
"""Trainium2 Bass kernel for GQA attention (B=2, S=2048, HID=2048, H=16, G=4, D=128).

Sharding: 8 cores = 2 batches x 4 kv-groups. Core c handles batch c//4 and
kv-group c%4 (4 q heads + 1 kv head). Wq/Wk/Wv column-sharded by head group,
Wo row-sharded; per-core partial outputs are summed on the host per batch
(the unshard / all-reduce step).
"""

import os
import sys

sys.path.insert(0, "/opt/trn_rl_repo")

import numpy as np

B, S, HID = 2, 2048, 2048
H, G, D = 16, 4, 128
REP = H // G  # q heads per kv head = 4
NEG = -1e30
SCALE = 1.0 / np.sqrt(np.float32(D))

NKT = HID // 128  # 16 contraction tiles for projections
NSC = S // 512  # 4 s-chunks of 512
NST = S // 128  # 16 s-tiles of 128


def _emit(nc, tc, aps):
    """Emit the per-core program. aps: dict of DRAM APs."""
    from contextlib import ExitStack

    from concourse import mybir

    f32 = mybir.dt.float32
    f32r = mybir.dt.float32r
    Exp = mybir.ActivationFunctionType.Exp

    def r(ap):
        return ap.bitcast(f32r)

    xT, cosT, sinT = aps["xT"], aps["cosT"], aps["sinT"]
    wq, wk, wv, wo = aps["wq"], aps["wk"], aps["wv"], aps["wo"]
    mask, ident = aps["mask"], aps["ident"]
    out = aps["out"]

    # DRAM views with 128-partition tiling of the contraction dim
    xT_t = xT.rearrange("(t p) s -> p t s", p=128)  # [128, 16, 2048]
    wq_t = wq.rearrange("(t p) m -> p t m", p=128)  # [128, 16, 512]
    wk_t = wk.rearrange("(t p) m -> p t m", p=128)  # [128, 16, 128]
    wv_t = wv.rearrange("(t p) m -> p t m", p=128)  # [128, 16, 128]
    wo_t = wo.rearrange("(t p) n -> p t n", p=128)  # [128, 4, 2048]

    with ExitStack() as ctx:
        persist = ctx.enter_context(tc.tile_pool(name="persist", bufs=1))

        # constants
        ones_t = persist.tile([128, 128], f32, tag="ones", name="ones")
        nc.gpsimd.dma_start(r(ones_t[:]), r(aps["ones"]))
        zeros_sb = persist.tile([128, 384], f32, tag="zeros", name="zeros_sb")
        nc.gpsimd.dma_start(r(zeros_sb[:]), r(aps["zeros"]))
        mask_t = persist.tile([128, 128], f32, tag="mask", name="mask_t")
        nc.gpsimd.dma_start(mask_t[:], mask)
        ident_t = persist.tile([128, 128], f32, tag="ident", name="ident_t")
        nc.gpsimd.dma_start(ident_t[:], ident)

        # persistent activations
        qt = [
            persist.tile([128, S], f32, tag=f"qt{h}", name=f"qt{h}")
            for h in range(REP)
        ]
        kt_sb = persist.tile([128, S], f32, tag="kt", name="kt_sb")
        v_sb = persist.tile([128, NST, 128], f32, tag="v", name="v_sb")

        # ---------------- Phase A: projections + RoPE ----------------
        with ExitStack() as actx:
            wpool = actx.enter_context(tc.tile_pool(name="wqkv", bufs=1))
            xpool = actx.enter_context(tc.tile_pool(name="xslab", bufs=3))
            ppool = actx.enter_context(tc.tile_pool(name="projps", bufs=6, space="PSUM"))
            rpool = actx.enter_context(tc.tile_pool(name="rope", bufs=2))
            tpool = actx.enter_context(tc.tile_pool(name="trig", bufs=1))
            vtp = actx.enter_context(tc.tile_pool(name="vtp", bufs=2, space="PSUM"))

            wq_sb = wpool.tile([128, NKT, 512], f32, tag="wq", name="wq_sb")
            wk_sb = wpool.tile([128, NKT, 128], f32, tag="wk", name="wk_sb")
            wv_sb = wpool.tile([128, NKT, 128], f32, tag="wv", name="wv_sb")
            # chunk-0 x slabs first (the dense-start gate), then weights in
            # arrival-order pieces, then trig tables.
            xs0 = [
                xpool.tile([128, 8, 512], f32, tag="x", name="xs0"),
                xpool.tile([128, 8, 512], f32, tag="x", name="xs0b"),
            ]
            nc.sync.dma_start(r(xs0[0][:]), r(xT_t[:, 0:8, 0:512]))
            nc.sync.dma_start(r(xs0[1][:]), r(xT_t[:, 8:16, 0:512]))
            for p4 in range(4):
                p4s = slice(p4 * 4, (p4 + 1) * 4)
                nc.sync.dma_start(r(wq_sb[:, p4s, :]), r(wq_t[:, p4s, :]))
                nc.sync.dma_start(r(wk_sb[:, p4s, :]), r(wk_t[:, p4s, :]))
                nc.sync.dma_start(r(wv_sb[:, p4s, :]), r(wv_t[:, p4s, :]))
            cos_sb = tpool.tile([128, S], f32, tag="cos", name="cos_sb")
            nc.sync.dma_start(cos_sb[:], cosT)
            sin_sb = tpool.tile([128, S], f32, tag="sin", name="sin_sb")
            nc.sync.dma_start(sin_sb[:], sinT)
            vt_sb = tpool.tile([128, S], f32, tag="vt", name="vt_sb")

            lo = slice(0, 64)
            hi = slice(64, 128)
            for c in range(NSC):
                cs = slice(c * 512, (c + 1) * 512)
                q_acc = [
                    ppool.tile([128, 512], f32, tag="acc", name="q_acc")
                    for _ in range(REP)
                ]
                k_acc = ppool.tile([128, 512], f32, tag="acc", name="k_acc")
                v_acc = ppool.tile([128, 512], f32, tag="acc", name="v_acc")
                for half in range(2):
                    if c == 0:
                        xs = xs0[half]
                    else:
                        xs = xpool.tile([128, 8, 512], f32, tag="x", name="xs")
                        for p2 in range(4):
                            p2s = slice(p2 * 2, (p2 + 1) * 2)
                            p2g = slice(half * 8 + p2 * 2, half * 8 + (p2 + 1) * 2)
                            nc.sync.dma_start(r(xs[:, p2s, :]), r(xT_t[:, p2g, cs]))
                    for t in range(8):
                        g = half * 8 + t
                        st = g == 0
                        sp = g == NKT - 1
                        rhs = r(xs[:, t, :])
                        for h in range(REP):
                            nc.tensor.matmul(
                                q_acc[h][:],
                                lhsT=r(wq_sb[:, g, h * 128 : (h + 1) * 128]),
                                rhs=rhs,
                                start=st,
                                stop=sp,
                            )
                        nc.tensor.matmul(
                            k_acc[:], lhsT=r(wk_sb[:, g, :]), rhs=rhs, start=st, stop=sp
                        )
                        nc.tensor.matmul(
                            v_acc[:], lhsT=r(wv_sb[:, g, :]), rhs=rhs, start=st, stop=sp
                        )
                # RoPE: dest = acc*cos + rot_half(acc)*sin
                for acc, dest in [(q_acc[h], qt[h]) for h in range(REP)] + [
                    (k_acc, kt_sb)
                ]:
                    tmp_a = rpool.tile([128, 512], f32, tag="tmpa", name="tmp_a")
                    tmp_b = rpool.tile([128, 512], f32, tag="tmpb", name="tmp_b")
                    nc.vector.tensor_mul(tmp_a[lo, :], acc[lo, :], cos_sb[lo, cs])
                    nc.vector.tensor_mul(tmp_a[hi, :], acc[hi, :], cos_sb[hi, cs])
                    nc.vector.tensor_mul(tmp_b[lo, :], acc[hi, :], sin_sb[lo, cs])
                    nc.vector.tensor_mul(tmp_b[hi, :], acc[lo, :], sin_sb[hi, cs])
                    nc.vector.tensor_sub(r(dest[lo, cs]), tmp_a[lo, :], tmp_b[lo, :])
                    nc.vector.tensor_add(r(dest[hi, cs]), tmp_a[hi, :], tmp_b[hi, :])
                nc.any.tensor_copy(vt_sb[:, cs], v_acc[:])

            # V: [d, s] -> [s, d] via PE transpose (A->B boundary filler)
            for i in range(NST):
                vps = vtp.tile([128, 128], f32, tag="vtp", name="vps")
                nc.tensor.transpose(
                    vps[:], vt_sb[:, i * 128 : (i + 1) * 128], ident_t[:]
                )
                nc.any.tensor_copy(r(v_sb[:, i, :]), vps[:])

        # ------------- Phases B+C (interleaved per chunk) -------------
        with ExitStack() as bcctx:
            wopool = bcctx.enter_context(tc.tile_pool(name="wop", bufs=1))
            wo_sb = wopool.tile([128, REP, HID], f32, tag="wo", name="wo_sb")
            for p4 in range(4):
                nc.sync.dma_start(r(wo_sb[:, p4, :]), r(wo_t[:, p4, :]))
            aopool = bcctx.enter_context(tc.tile_pool(name="aop", bufs=1))
            aot = [
                aopool.tile([128, S], f32, tag=f"aot{h}", name=f"aot{h}")
                for h in range(REP)
            ]

            if True:
                epool = bcctx.enter_context(tc.tile_pool(name="eslab", bufs=5))
                spool = bcctx.enter_context(tc.tile_pool(name="scps", bufs=2, space="PSUM"))
                apool = bcctx.enter_context(tc.tile_pool(name="accps", bufs=2, space="PSUM"))
                opool = bcctx.enter_context(tc.tile_pool(name="outps", bufs=2, space="PSUM"))
                ocpool = bcctx.enter_context(tc.tile_pool(name="ocopy", bufs=4))
                rcpool = bcctx.enter_context(tc.tile_pool(name="recip", bufs=3))

                for c in range(NSC):
                    qs = slice(c * 512, (c + 1) * 512)
                    for h in range(REP):
                        av = apool.tile([128, 512], f32, tag="av", name="av")
                        den = apool.tile([128, 512], f32, tag="den", name="den")
                        for kb in range(c + 1):
                            es = epool.tile([128, 4, 512], f32, tag="es", name="es")
                            for j in range(4):
                                i = kb * 4 + j
                                sp_t = spool.tile([128, 512], f32, tag="sc", name="sp_t")
                                j0 = j * 128 if kb == c else 0
                                nc.tensor.matmul(
                                    sp_t[:, j0:512],
                                    lhsT=r(kt_sb[:, i * 128 : (i + 1) * 128]),
                                    rhs=r(qt[h][:, c * 512 + j0 : (c + 1) * 512]),
                                    start=True,
                                    stop=True,
                                )
                                if kb == c:
                                    # diagonal block: q sub-block j partially masked
                                    nc.vector.tensor_add(
                                        sp_t[:, j * 128 : (j + 1) * 128],
                                        sp_t[:, j * 128 : (j + 1) * 128],
                                        mask_t[:],
                                    )
                                nc.scalar.activation(
                                    r(es[:, j, j0:512]),
                                    sp_t[:, j0:512],
                                    Exp,
                                    scale=float(SCALE),
                                )
                            for j in range(4):
                                i = kb * 4 + j
                                st = i == 0
                                sp = i == 4 * c + 3
                                j0 = j * 128 if kb == c else 0
                                nc.tensor.matmul(
                                    av[:, j0:512],
                                    lhsT=r(v_sb[:, i, :]),
                                    rhs=r(es[:, j, j0:512]),
                                    start=st,
                                    stop=sp,
                                )
                                nc.tensor.matmul(
                                    den[:, j0:512],
                                    lhsT=r(ones_t[:]),
                                    rhs=r(es[:, j, j0:512]),
                                    start=st,
                                    stop=sp,
                                )
                        rc = rcpool.tile([128, 512], f32, tag="rc", name="rc")
                        nc.vector.reciprocal_approx_fast(rc[:], den[:])
                        nc.vector.tensor_mul(r(aot[h][:, qs]), av[:], rc[:])

                    # output projection for the s-tiles of this chunk
                    for st_i in range(4 * c, 4 * c + 4):
                        ss = slice(st_i * 128, (st_i + 1) * 128)
                        for hc in range(NSC):
                            hs = slice(hc * 512, (hc + 1) * 512)
                            ops = opool.tile([128, 512], f32, tag="o", name="ops")
                            for m in range(REP):
                                nc.tensor.matmul(
                                    ops[:],
                                    lhsT=r(aot[m][:, ss]),
                                    rhs=r(wo_sb[:, m, hs]),
                                    start=(m == 0),
                                    stop=(m == REP - 1),
                                )
                            oc = ocpool.tile([128, 512], f32, tag="oc", name="oc")
                            nc.any.tensor_copy(oc[:], ops[:])
                            nc.sync.dma_start(out[ss, hs], oc[:])


def build_program():
    import concourse.tile as tile
    from concourse import bacc, mybir

    f32 = mybir.dt.float32
    nc = bacc.Bacc("TRN2", target_bir_lowering=False, debug=False, num_devices=8)
    aps = {}
    aps["xT"] = nc.dram_tensor("xT", [HID, S], f32, kind="ExternalInput").ap()
    aps["cosT"] = nc.dram_tensor("cosT", [D, S], f32, kind="ExternalInput").ap()
    aps["sinT"] = nc.dram_tensor("sinT", [D, S], f32, kind="ExternalInput").ap()
    aps["wq"] = nc.dram_tensor("wq", [HID, REP * D], f32, kind="ExternalInput").ap()
    aps["wk"] = nc.dram_tensor("wk", [HID, D], f32, kind="ExternalInput").ap()
    aps["wv"] = nc.dram_tensor("wv", [HID, D], f32, kind="ExternalInput").ap()
    aps["wo"] = nc.dram_tensor("wo", [REP * D, HID], f32, kind="ExternalInput").ap()
    aps["mask"] = nc.dram_tensor("mask", [128, 128], f32, kind="ExternalInput").ap()
    aps["ones"] = nc.dram_tensor("ones", [128, 128], f32, kind="ExternalInput").ap()
    aps["zeros"] = nc.dram_tensor("zeros", [128, 384], f32, kind="ExternalInput").ap()
    aps["ident"] = nc.dram_tensor("ident", [128, 128], f32, kind="ExternalInput").ap()
    aps["out"] = nc.dram_tensor("out", [S, HID], f32, kind="ExternalOutput").ap()

    with tile.TileContext(nc) as tc:
        _emit(nc, tc, aps)
    nc.compile()
    return nc


def round_fp32r(a):
    """Round-to-nearest-even to fp32r (fp32 with low 12 mantissa bits dropped)."""
    u = np.ascontiguousarray(a, dtype=np.float32).view(np.uint32).astype(np.uint64)
    lsb = (u >> 12) & 1
    u = u + 0x7FF + lsb
    u = (u >> 12) << 12
    return (u & 0xFFFFFFFF).astype(np.uint32).view(np.float32)


def make_in_maps(x, cos, sin, Wq, Wk, Wv, Wo):
    """Build the 8 per-core input dicts. Core c: batch c//4, kv-group c%4."""
    mask = np.where(
        np.arange(128)[:, None] <= np.arange(128)[None, :], 0.0, NEG
    ).astype(np.float32)
    ident = np.eye(128, dtype=np.float32)
    ones = np.ones((128, 128), dtype=np.float32)
    zeros = np.zeros((128, 384), dtype=np.float32)
    xT = [round_fp32r(x[b].T) for b in range(B)]
    cosT = np.ascontiguousarray(cos.T)
    sinT = np.ascontiguousarray(sin.T)
    in_maps = []
    for c in range(8):
        b, g = c // 4, c % 4
        in_maps.append(
            {
                "xT": xT[b],
                "cosT": cosT,
                "sinT": sinT,
                "wq": round_fp32r(Wq[:, g * REP * D : (g + 1) * REP * D]),
                "wk": round_fp32r(Wk[:, g * D : (g + 1) * D]),
                "wv": round_fp32r(Wv[:, g * D : (g + 1) * D]),
                "wo": round_fp32r(Wo[g * REP * D : (g + 1) * REP * D, :]),
                "mask": mask,
                "ident": ident,
                "ones": ones,
                "zeros": zeros,
            }
        )
    return in_maps


def kernel(x, cos, sin, Wq, Wk, Wv, Wo):
    from concourse import bass_utils

    nc = build_program()
    in_maps = make_in_maps(x, cos, sin, Wq, Wk, Wv, Wo)
    trace = bool(int(os.environ.get("BASS_KERNEL_TRACE", "0")))
    res = bass_utils.run_bass_kernel_spmd(
        nc,
        in_maps,
        core_ids=list(range(8)),
        trace=trace,
    )
    if trace:
        print(f"HW exec time: {res.exec_time_ns} ns")
        if res.instructions_and_trace is not None:
            print(f"trace: {res.instructions_and_trace[1]}")
    out = np.empty((B, S, HID), dtype=np.float32)
    for b in range(B):
        acc = res.results[4 * b]["out"].astype(np.float32).copy()
        for g in range(1, G):
            acc += res.results[4 * b + g]["out"]
        out[b] = acc
    return out



# revision 3
# speedup vs baseline: 1.1819x; 1.1819x over previous
"""Trainium2 Bass kernel for GQA attention (B=2, S=2048, HID=2048, H=16, G=4, D=128).

Sharding: 8 cores = 2 batches x 4 kv-groups. Core c handles batch c//4 and
kv-group c%4 (4 q heads + 1 kv head). Wq/Wk/Wv column-sharded by head group,
Wo row-sharded; per-core partial outputs are summed on the host per batch
(the unshard / all-reduce step).

v2: bf16 operands, fully pipelined per-chunk schedule
(proj_c -> attn_c -> outproj_c with proj passes ordered k/v first),
V projected directly in transposed layout (lhsT = x), full-partition RoPE
with sign-folded sin table, fine-grained startup DMA, bf16 output.
"""

import os
import sys

sys.path.insert(0, "/opt/trn_rl_repo")

import numpy as np

B, S, HID = 2, 2048, 2048
H, G, D = 16, 4, 128
REP = H // G  # q heads per kv head = 4
NEG = -1e30
SCALE = 1.0 / np.sqrt(np.float32(D))

NKT = HID // 128  # 16 contraction tiles for projections
NSC = S // 512  # 4 s-chunks of 512
NST = S // 128  # 16 s-tiles of 128


def _emit(nc, tc, aps):
    """Emit the per-core program. aps: dict of DRAM APs."""
    from contextlib import ExitStack

    from concourse import mybir

    f32 = mybir.dt.float32
    bf16 = mybir.dt.bfloat16
    Exp = mybir.ActivationFunctionType.Exp

    xT, cosT, sinT = aps["xT"], aps["cosT"], aps["sinT"]
    wq, wk, wv, wo = aps["wq"], aps["wk"], aps["wv"], aps["wo"]
    mask, ones = aps["mask"], aps["ones"]
    out = aps["out"]

    # DRAM views with 128-partition tiling of the contraction dim
    xT_t = xT.rearrange("(t p) s -> p t s", p=128)  # [128, 16, 2048] bf16
    wq_t = wq.rearrange("(t p) m -> p t m", p=128)  # [128, 16, 512]
    wk_t = wk.rearrange("(t p) m -> p t m", p=128)  # [128, 16, 128]
    wv_t = wv.rearrange("(t p) m -> p t m", p=128)  # [128, 16, 128]
    wo_t = wo.rearrange("(t p) n -> p t n", p=128)  # [128, 4, 2048]

    lo = slice(0, 64)
    hi = slice(64, 128)

    with ExitStack() as ctx:
        persist = ctx.enter_context(tc.tile_pool(name="persist", bufs=1))

        # constants
        mask_t = persist.tile([128, 128], f32, tag="mask", name="mask_t")
        ones_t = persist.tile([128, 128], bf16, tag="ones", name="ones_t")

        # weight pieces (separate tiles => fine-grained DMA deps)
        wq_p = [
            persist.tile([128, 4, 512], bf16, tag=f"wq{i}", name=f"wq{i}")
            for i in range(4)
        ]
        wk_p = [
            persist.tile([128, 8, 128], bf16, tag=f"wk{i}", name=f"wk{i}")
            for i in range(2)
        ]
        wv_p = [
            persist.tile([128, 8, 128], bf16, tag=f"wv{i}", name=f"wv{i}")
            for i in range(2)
        ]
        wo_p = [
            persist.tile([128, 1, 2048], bf16, tag=f"wo{i}", name=f"wo{i}")
            for i in range(4)
        ]
        cos_p = [
            persist.tile([128, 1024], f32, tag=f"cos{i}", name=f"cos{i}")
            for i in range(2)
        ]
        sin_p = [
            persist.tile([128, 1024], f32, tag=f"sin{i}", name=f"sin{i}")
            for i in range(2)
        ]

        # persistent activations
        qt = [
            persist.tile([128, S], bf16, tag=f"qt{h}", name=f"qt{h}")
            for h in range(REP)
        ]
        kt = persist.tile([128, S], bf16, tag="kt", name="kt")
        vs = persist.tile([128, NST, 128], bf16, tag="vs", name="vs")
        aot = [
            persist.tile([128, S], bf16, tag=f"aot{h}", name=f"aot{h}")
            for h in range(REP)
        ]

        xpool = ctx.enter_context(tc.tile_pool(name="xsl", bufs=8))
        quad = ctx.enter_context(tc.tile_pool(name="quad", bufs=4, space="PSUM"))
        pb = ctx.enter_context(tc.tile_pool(name="pb", bufs=4, space="PSUM"))
        epool = ctx.enter_context(tc.tile_pool(name="es", bufs=4))
        rpool = ctx.enter_context(tc.tile_pool(name="rope", bufs=4))
        rcp = ctx.enter_context(tc.tile_pool(name="rc", bufs=2))
        ocp = ctx.enter_context(tc.tile_pool(name="oc", bufs=4))

        # ---------------- startup DMA sequence ----------------
        # x slabs: per chunk, 4 piece-tiles of [128, 4, 512]
        xs = [
            [
                xpool.tile([128, 4, 512], bf16, tag="x", name=f"xs{c}_{p}")
                for p in range(4)
            ]
            for c in range(NSC)
        ]

        def dma_x_chunk(c, eng):
            for p in range(4):
                eng.dma_start(
                    xs[c][p][:], xT_t[:, 4 * p : 4 * p + 4, c * 512 : (c + 1) * 512]
                )

        # gpsimd: consts, wk, wv, then wo (needed late)
        nc.gpsimd.dma_start(mask_t[:], mask)
        nc.gpsimd.dma_start(ones_t[:], ones)
        nc.gpsimd.dma_start(wk_p[0][:], wk_t[:, 0:8, :])
        nc.gpsimd.dma_start(wk_p[1][:], wk_t[:, 8:16, :])
        nc.gpsimd.dma_start(wv_p[0][:], wv_t[:, 0:8, :])
        nc.gpsimd.dma_start(wv_p[1][:], wv_t[:, 8:16, :])
        # sync: x chunk0 then wq
        dma_x_chunk(0, nc.sync)
        for i in range(4):
            nc.sync.dma_start(wq_p[i][:], wq_t[:, 4 * i : 4 * i + 4, :])
        # scalar: trig tables (first the pieces for chunks 0-1)
        nc.scalar.dma_start(cos_p[0][:], cosT[:, 0:1024])
        nc.scalar.dma_start(sin_p[0][:], sinT[:, 0:1024])
        nc.scalar.dma_start(cos_p[1][:], cosT[:, 1024:2048])
        nc.scalar.dma_start(sin_p[1][:], sinT[:, 1024:2048])
        # gpsimd tail: wo pieces
        for m in range(4):
            nc.gpsimd.dma_start(wo_p[m][:], wo_t[:, m : m + 1, :])
        # prefetch x chunk 1 early
        dma_x_chunk(1, nc.gpsimd)

        def rope(acc, dest, cs_off, piece):
            """dest = acc*cos + swap_half(acc)*sin_signed  (sin pre-negated in
            its low half on the host)."""
            cp = cos_p[piece]
            sp_ = sin_p[piece]
            co = slice(cs_off, cs_off + 512)
            tmp_a = rpool.tile([128, 512], f32, tag="ra", name="tmp_a")
            tmp_b = rpool.tile([128, 512], f32, tag="rb", name="tmp_b")
            nc.vector.tensor_mul(tmp_b[lo, :], acc[hi, :], sp_[lo, co])
            nc.vector.tensor_mul(tmp_b[hi, :], acc[lo, :], sp_[hi, co])
            nc.vector.tensor_mul(tmp_a[:], acc[:], cp[:, co])
            nc.vector.tensor_add(dest, tmp_a[:], tmp_b[:])

        for c in range(NSC):
            cs = slice(c * 512, (c + 1) * 512)
            piece = c // 2
            cs_off = (c % 2) * 512

            # ---------------- projections ----------------
            # pass A: k and v^T (so RoPE-k and V land first for attn)
            k_acc = quad.tile([128, 512], f32, tag="qd", name="k_acc")
            for t in range(NKT):
                nc.tensor.matmul(
                    k_acc[:],
                    lhsT=wk_p[t // 8][:, t % 8, :],
                    rhs=xs[c][t // 4][:, t % 4, :],
                    start=(t == 0),
                    stop=(t == NKT - 1),
                )
            vT_acc = quad.tile([128, 4, 128], f32, tag="qd", name="vT_acc")
            for i in range(4):
                for t in range(NKT):
                    nc.tensor.matmul(
                        vT_acc[:, i, :],
                        lhsT=xs[c][t // 4][:, t % 4, i * 128 : (i + 1) * 128],
                        rhs=wv_p[t // 8][:, t % 8, :],
                        start=(t == 0),
                        stop=(t == NKT - 1),
                    )
            rope(k_acc, kt[:, cs], cs_off, piece)
            nc.scalar.copy(vs[:, 4 * c : 4 * c + 4, :], vT_acc[:])

            # pass B: q0, q1 ; pass C: q2, q3
            for pair in range(2):
                q_acc = [
                    quad.tile([128, 512], f32, tag="qd", name=f"q_acc{pair}{j}")
                    for j in range(2)
                ]
                for t in range(NKT):
                    for j in range(2):
                        h = 2 * pair + j
                        nc.tensor.matmul(
                            q_acc[j][:],
                            lhsT=wq_p[t // 4][:, t % 4, h * 128 : (h + 1) * 128],
                            rhs=xs[c][t // 4][:, t % 4, :],
                            start=(t == 0),
                            stop=(t == NKT - 1),
                        )
                for j in range(2):
                    h = 2 * pair + j
                    rope(q_acc[j], qt[h][:, cs], cs_off, piece)

            # prefetch x for chunk c+2
            if c + 2 < NSC:
                dma_x_chunk(c + 2, nc.sync)

            # ---------------- attention for q-chunk c ----------------
            for h in range(REP):
                av = pb.tile([128, 512], f32, tag="pb", name="av")
                den = pb.tile([128, 512], f32, tag="pb", name="den")
                for kb in range(c + 1):
                    es = epool.tile([128, 4, 512], bf16, tag="es", name="es")
                    for j in range(4):
                        i = kb * 4 + j
                        j0 = j * 128 if kb == c else 0
                        sp_t = quad.tile([128, 512], f32, tag="qd", name="sp_t")
                        nc.tensor.matmul(
                            sp_t[:, j0:512],
                            lhsT=kt[:, i * 128 : (i + 1) * 128],
                            rhs=qt[h][:, c * 512 + j0 : (c + 1) * 512],
                            start=True,
                            stop=True,
                        )
                        if kb == c:
                            nc.vector.tensor_add(
                                sp_t[:, j * 128 : (j + 1) * 128],
                                sp_t[:, j * 128 : (j + 1) * 128],
                                mask_t[:],
                            )
                        nc.scalar.activation(
                            es[:, j, j0:512],
                            sp_t[:, j0:512],
                            Exp,
                            scale=float(SCALE),
                        )
                    for j in range(4):
                        i = kb * 4 + j
                        st = i == 0
                        sp = i == 4 * c + 3
                        j0 = j * 128 if kb == c else 0
                        nc.tensor.matmul(
                            av[:, j0:512],
                            lhsT=vs[:, i, :],
                            rhs=es[:, j, j0:512],
                            start=st,
                            stop=sp,
                        )
                        nc.tensor.matmul(
                            den[:, j0:512],
                            lhsT=ones_t[:],
                            rhs=es[:, j, j0:512],
                            start=st,
                            stop=sp,
                        )
                rc = rcp.tile([128, 512], f32, tag="rc", name="rc")
                nc.vector.reciprocal_approx_fast(rc[:], den[:])
                nc.vector.tensor_mul(aot[h][:, cs], av[:], rc[:])

            # ---------------- output projection for chunk c ----------------
            for st_i in range(4 * c, 4 * c + 4):
                ss = slice(st_i * 128, (st_i + 1) * 128)
                for hc in range(NSC):
                    hs = slice(hc * 512, (hc + 1) * 512)
                    ops = pb.tile([128, 512], f32, tag="pb", name="ops")
                    for m in range(REP):
                        nc.tensor.matmul(
                            ops[:],
                            lhsT=aot[m][:, ss],
                            rhs=wo_p[m][:, 0, hs],
                            start=(m == 0),
                            stop=(m == REP - 1),
                        )
                    oc = ocp.tile([128, 512], bf16, tag="oc", name="oc")
                    if (st_i + hc) % 2 == 0:
                        nc.scalar.copy(oc[:], ops[:])
                    else:
                        nc.vector.tensor_copy(oc[:], ops[:])
                    nc.sync.dma_start(out[ss, hs], oc[:])


def build_program():
    import concourse.tile as tile
    from concourse import bacc, mybir

    f32 = mybir.dt.float32
    bf16 = mybir.dt.bfloat16
    nc = bacc.Bacc("TRN2", target_bir_lowering=False, debug=False, num_devices=8)
    aps = {}
    aps["xT"] = nc.dram_tensor("xT", [HID, S], bf16, kind="ExternalInput").ap()
    aps["cosT"] = nc.dram_tensor("cosT", [D, S], f32, kind="ExternalInput").ap()
    aps["sinT"] = nc.dram_tensor("sinT", [D, S], f32, kind="ExternalInput").ap()
    aps["wq"] = nc.dram_tensor("wq", [HID, REP * D], bf16, kind="ExternalInput").ap()
    aps["wk"] = nc.dram_tensor("wk", [HID, D], bf16, kind="ExternalInput").ap()
    aps["wv"] = nc.dram_tensor("wv", [HID, D], bf16, kind="ExternalInput").ap()
    aps["wo"] = nc.dram_tensor("wo", [REP * D, HID], bf16, kind="ExternalInput").ap()
    aps["mask"] = nc.dram_tensor("mask", [128, 128], f32, kind="ExternalInput").ap()
    aps["ones"] = nc.dram_tensor("ones", [128, 128], bf16, kind="ExternalInput").ap()
    aps["out"] = nc.dram_tensor("out", [S, HID], bf16, kind="ExternalOutput").ap()

    with tile.TileContext(nc) as tc:
        _emit(nc, tc, aps)
    nc.compile()
    return nc


def make_in_maps(x, cos, sin, Wq, Wk, Wv, Wo):
    """Build the 8 per-core input dicts. Core c: batch c//4, kv-group c%4."""
    import ml_dtypes

    bf = ml_dtypes.bfloat16
    mask = np.where(
        np.arange(128)[:, None] <= np.arange(128)[None, :], 0.0, NEG
    ).astype(np.float32)
    ones = np.ones((128, 128), dtype=bf)
    xT = [np.ascontiguousarray(x[b].T).astype(bf) for b in range(B)]
    cosT = np.ascontiguousarray(cos.T).astype(np.float32)
    sinT = np.ascontiguousarray(sin.T).astype(np.float32)
    sinT[0:64, :] *= -1.0  # sign-fold rotate_half's negation into the table
    in_maps = []
    for c in range(8):
        b, g = c // 4, c % 4
        in_maps.append(
            {
                "xT": xT[b],
                "cosT": cosT,
                "sinT": sinT,
                "wq": Wq[:, g * REP * D : (g + 1) * REP * D].astype(bf),
                "wk": Wk[:, g * D : (g + 1) * D].astype(bf),
                "wv": Wv[:, g * D : (g + 1) * D].astype(bf),
                "wo": Wo[g * REP * D : (g + 1) * REP * D, :].astype(bf),
                "mask": mask,
                "ones": ones,
            }
        )
    return in_maps


def kernel(x, cos, sin, Wq, Wk, Wv, Wo):
    from concourse import bass_utils

    nc = build_program()
    in_maps = make_in_maps(x, cos, sin, Wq, Wk, Wv, Wo)
    trace = bool(int(os.environ.get("BASS_KERNEL_TRACE", "0")))
    res = bass_utils.run_bass_kernel_spmd(
        nc,
        in_maps,
        core_ids=list(range(8)),
        trace=trace,
    )
    if trace:
        print(f"HW exec time: {res.exec_time_ns} ns")
        if res.instructions_and_trace is not None:
            print(f"trace: {res.instructions_and_trace[1]}")
    out = np.empty((B, S, HID), dtype=np.float32)
    for b in range(B):
        acc = res.results[4 * b]["out"].astype(np.float32)
        for g in range(1, G):
            acc = acc + res.results[4 * b + g]["out"].astype(np.float32)
        out[b] = acc
    return out
